# revision 5
# baseline (speedup 1.0000x reference)
"""Causal self-attention (B=2, T=2048, C=1024, NH=16, HD=64) on 8 TRN2 cores.

Sharding: core c -> batch b = c//4, head group j = c%4 (4 heads: 4j..4j+3).
Each core computes its batch's QKV projection for its 4 heads, rope, causal
flash-style attention in S^T layout (k on partitions, q on free dim), and a
partial output projection y_part^T = Wp_slice^T.T @ attT. Host sums the 4
per-batch partials and adds b_proj.

Device layouts (per core, t = 2048 tokens of its batch):
  xT   [128, 8, 2048]  bf16   x[b].T tiled over 8 c-tiles
  q/k  [128, 2, 2048]  bf16   head-pair dims on partitions, rope applied
  v    [128, 64, 65]   bf16   [tok-part, (tt,hp,h), 64 dims + ones col]
  S^T  psum [128, 2048] f32   4 tiles: [kt0h0|kt0h1|kt1h0|kt1h1]
  P^T  [128, 2048] bf16       exp(S^T/8), causal-masked
  PV   psum [65, 512]         rows 0-63 att^T, row 64 rowsum (ones col)
  attT [128, 2, 2048] f32 -> normalized bf16 -> y^T = WpT.T @ attbf
"""
import numpy as np
import ml_dtypes
from contextlib import ExitStack

import concourse.bass as bass
import concourse.mybir as mybir
import concourse.tile as tile
from concourse import bacc
from concourse.bass_utils import run_bass_kernel_spmd

F32 = mybir.dt.float32
BF16 = mybir.dt.bfloat16
AF = mybir.ActivationFunctionType

B, T, C = 2, 2048, 1024
NH, HD = 16, 64
TL = 2048          # per-core token count (one batch)
NCT = C // 128     # 8 contraction tiles
NTC = TL // 512    # 4 t-chunks of 512
NTT = TL // 128    # 16 token tiles of 128

TRACE = False      # set by test harness for profiling runs
_CACHE = {}


def _build_nc():
    nc = bacc.Bacc("TRN2", target_bir_lowering=False, debug=False)
    xT_d = nc.dram_tensor("xT", [128, NCT, TL], BF16, kind="ExternalInput").ap()
    wqk_d = nc.dram_tensor("wqkT", [128, NCT, 512], BF16, kind="ExternalInput").ap()
    wv_d = nc.dram_tensor("wvT", [128, NCT, 256], BF16, kind="ExternalInput").ap()
    bqk_d = nc.dram_tensor("bqk", [128, 4], F32, kind="ExternalInput").ap()
    bv_d = nc.dram_tensor("bv", [128, 256], F32, kind="ExternalInput").ap()
    rope_d = nc.dram_tensor("rope", [128, TL], F32, kind="ExternalInput").ap()
    masks_d = nc.dram_tensor("masks", [128, 4, 1024], BF16, kind="ExternalInput").ap()
    wp_d = nc.dram_tensor("wpT", [128, 2, 1024], BF16, kind="ExternalInput").ap()
    yT_d = nc.dram_tensor("yT", [1024, TL], F32, kind="ExternalOutput").ap()
    rs_dram = nc.dram_tensor("rs_scratch", [4, TL], F32)

    with tile.TileContext(nc) as tc, ExitStack() as ctx:
        sb = ctx.enter_context(tc.tile_pool(name="sb", bufs=1))
        tmps = ctx.enter_context(tc.tile_pool(name="tmps", bufs=4))
        ptp = ctx.enter_context(tc.tile_pool(name="ptp", bufs=3))
        ytp = ctx.enter_context(tc.tile_pool(name="ytp", bufs=4))

        xT = sb.tile([128, NCT, TL], BF16)
        wqk = sb.tile([128, NCT, 512], BF16)
        wv = sb.tile([128, NCT, 256], BF16)
        bqk = sb.tile([128, 4], F32)
        bv = sb.tile([128, 256], F32)
        rope = sb.tile([128, TL], F32)
        masks = sb.tile([128, 4, 1024], BF16)
        wp = sb.tile([128, 2, 1024], BF16)
        q_sb = sb.tile([128, 2, TL], BF16)
        k_sb = sb.tile([128, 2, TL], BF16)
        v_sb = sb.tile([128, 4 * NTT, 65], BF16)
        attT = sb.tile([128, 2, TL], F32)
        attbf = sb.tile([128, 2, TL], BF16)
        rs_sb = sb.tile([128, TL], F32)   # rowsum for unit u at partition 32*u
        rsr_sb = sb.tile([128, TL], F32)
        rsb_sb = sb.tile([128, 2, TL], F32)

        # input DMAs (xT split per c-tile for queue parallelism)
        for ct in range(NCT):
            nc.sync.dma_start(out=xT[:, ct, :], in_=xT_d[:, ct, :])
        nc.sync.dma_start(out=wqk, in_=wqk_d)
        nc.sync.dma_start(out=wv, in_=wv_d)
        nc.sync.dma_start(out=bqk, in_=bqk_d)
        nc.sync.dma_start(out=bv, in_=bv_d)
        nc.sync.dma_start(out=rope, in_=rope_d)
        nc.sync.dma_start(out=masks, in_=masks_d)
        nc.sync.dma_start(out=wp, in_=wp_d)
        nc.vector.memset(v_sb[:, :, 64:65], 1.0)

        with tc.tile_pool(name="pb", bufs=1, space="PSUM") as pb, \
             tc.tile_pool(name="pc", bufs=2, space="PSUM") as pc:
            # ---- Phase B: qkT = WqkT.T @ xT, +bias, *rope -> q_sb / k_sb ----
            for m in range(4):
                pss = [pb.tile([128, 512], F32, tag=f"pb{i}", name=f"pb_{m}_{i}")
                       for i in range(NTC)]
                for ct in range(NCT):
                    lhsT = wqk[:, ct, m * 128:(m + 1) * 128]
                    for tc4 in range(NTC):
                        nc.tensor.matmul(
                            pss[tc4], lhsT, xT[:, ct, tc4 * 512:(tc4 + 1) * 512],
                            start=(ct == 0), stop=(ct == NCT - 1),
                            skip_group_check=True)
                dest = q_sb if m < 2 else k_sb
                hp = m % 2
                for tc4 in range(NTC):
                    sl = slice(tc4 * 512, (tc4 + 1) * 512)
                    tmp = tmps.tile([128, 512], F32, tag="btmp")
                    nc.vector.tensor_scalar_add(tmp, pss[tc4], bqk[:, m:m + 1])
                    nc.vector.tensor_mul(dest[:, hp, sl], tmp, rope[:, sl])

            # ---- Phase C: v = xT.T @ WvT + bv (natural layout, ones col) ----
            for tt in range(NTT):
                ps = pc.tile([128, 256], F32, tag="pc")
                for ct in range(NCT):
                    nc.tensor.matmul(
                        ps, xT[:, ct, tt * 128:(tt + 1) * 128], wv[:, ct, :],
                        start=(ct == 0), stop=(ct == NCT - 1),
                        skip_group_check=True)
                for hp in range(2):
                    for h in range(2):
                        u = (tt * 2 + hp) * 2 + h
                        f0 = hp * 128 + h * 64
                        nc.vector.tensor_add(
                            v_sb[:, u, 0:64], ps[:, f0:f0 + 64], bv[:, f0:f0 + 64])

        # ---- Phase D: attention in S^T layout ----
        with tc.tile_pool(name="pds", bufs=1, space="PSUM") as pds, \
             tc.tile_pool(name="pdv", bufs=2, space="PSUM") as pdv:
            for qc in range(4):
                qsl = slice(qc * 512, (qc + 1) * 512)
                for hp in range(2):
                    pv = [pdv.tile([65, 512], F32, tag=f"pv{h}", name=f"pv_{qc}_{hp}_{h}")
                          for h in range(2)]
                    n_kt = 4 * (qc + 1)
                    for g in range(n_kt // 2):
                        sps = pds.tile([128, 2048], F32, tag="sps")
                        for i in range(2):
                            kt = 2 * g + i
                            ksl = slice(kt * 128, (kt + 1) * 128)
                            for h in range(2):
                                hsl = slice(h * 64, (h + 1) * 64)
                                nc.tensor.matmul(
                                    sps[:, (i * 2 + h) * 512:(i * 2 + h + 1) * 512],
                                    k_sb[hsl, hp, ksl], q_sb[hsl, hp, qsl])
                        pt = ptp.tile([128, 2048], BF16, tag="pt")
                        nc.scalar.activation(pt, sps, AF.Exp, bias=0.0, scale=0.125)
                        for i in range(2):
                            kt = 2 * g + i
                            if kt >= 4 * qc:  # partial (diagonal) tile: mask
                                d = kt - 4 * qc
                                nc.vector.tensor_mul(
                                    pt[:, i * 1024:(i + 1) * 1024],
                                    pt[:, i * 1024:(i + 1) * 1024],
                                    masks[:, d, :])
                        for i in range(2):
                            kt = 2 * g + i
                            for h in range(2):
                                u = (kt * 2 + hp) * 2 + h
                                nc.tensor.matmul(
                                    pv[h], v_sb[:, u, :],
                                    pt[:, (i * 2 + h) * 512:(i * 2 + h + 1) * 512],
                                    start=(kt == 0), stop=(kt == n_kt - 1),
                                    skip_group_check=True)
                    for h in range(2):
                        u4 = 2 * hp + h
                        nc.vector.tensor_copy(
                            attT[h * 64:(h + 1) * 64, hp, qsl], pv[h][0:64, :])
                        nc.vector.tensor_copy(
                            rs_sb[32 * u4:32 * u4 + 1, qsl], pv[h][64:65, :])

        # ---- rowsum normalize: reciprocal, DRAM-bounce broadcast, multiply ----
        for u4 in range(4):
            nc.vector.reciprocal(
                rsr_sb[32 * u4:32 * u4 + 1, :], rs_sb[32 * u4:32 * u4 + 1, :])
            nc.sync.dma_start(
                out=rs_dram[u4:u4 + 1, :], in_=rsr_sb[32 * u4:32 * u4 + 1, :])
        for hp in range(2):
            for h in range(2):
                u4 = 2 * hp + h
                bc_ap = bass.AP(tensor=rs_dram, offset=u4 * TL, ap=[[0, 64], [1, TL]])
                nc.sync.dma_start(out=rsb_sb[h * 64:(h + 1) * 64, hp, :], in_=bc_ap)
        for hp in range(2):
            nc.vector.tensor_mul(attbf[:, hp, :], attT[:, hp, :], rsb_sb[:, hp, :])

        # ---- Phase E: yT = WpT.T @ attbf ----
        with tc.tile_pool(name="pe", bufs=4, space="PSUM") as pe:
            for mt in range(8):
                pss = [pe.tile([128, 512], F32, tag=f"pe{i % 2}", name=f"pe_{mt}_{i}")
                       for i in range(NTC)]
                for hp in range(2):
                    lhsT = wp[:, hp, mt * 128:(mt + 1) * 128]
                    for tc4 in range(NTC):
                        nc.tensor.matmul(
                            pss[tc4], lhsT, attbf[:, hp, tc4 * 512:(tc4 + 1) * 512],
                            start=(hp == 0), stop=(hp == 1),
                            skip_group_check=True)
                for tc4 in range(NTC):
                    yt = ytp.tile([128, 512], F32, tag="yt")
                    nc.vector.tensor_copy(yt, pss[tc4])
                    nc.sync.dma_start(
                        out=yT_d[mt * 128:(mt + 1) * 128, tc4 * 512:(tc4 + 1) * 512],
                        in_=yt)
    nc.compile()
    return nc


def _rope_T():
    theta = 1.0 / (10000.0 ** (2.0 * np.arange(0, HD // 2, dtype=np.float32) / HD))
    seq = np.arange(1, T + 1, dtype=np.float32)
    ang = np.einsum('n,d->nd', seq, theta)
    ang = np.concatenate([ang, ang], axis=-1)
    f = (np.cos(ang) + np.sin(ang)).astype(np.float32)  # [T, 64]
    return np.concatenate([f.T, f.T], axis=0)           # [128, T]


def _host_inputs(x, W_attn, b_attn, W_proj, b_proj):
    bf = ml_dtypes.bfloat16
    ropeT = _rope_T()
    masks = np.empty((128, 4, 1024), dtype=bf)
    kp = np.arange(128)[:, None]
    qf = np.arange(512)[None, :]
    for d in range(4):
        m = ((kp + 128 * d) <= qf).astype(np.float32)
        masks[:, d, :] = np.concatenate([m, m], axis=1).astype(bf)

    in_maps = []
    for c in range(8):
        b, j = divmod(c, 4)
        hs = [4 * j + i for i in range(4)]
        xT = np.ascontiguousarray(x[b].T).astype(bf)          # [1024, TL]
        q_rows = np.concatenate([W_attn[64 * h:64 * (h + 1)] for h in hs], 0)
        k_rows = np.concatenate([W_attn[C + 64 * h:C + 64 * (h + 1)] for h in hs], 0)
        WqkT = np.concatenate([q_rows, k_rows], 0).T          # [1024, 512]
        bqk = np.concatenate(
            [np.concatenate([b_attn[64 * h:64 * (h + 1)] for h in hs]),
             np.concatenate([b_attn[C + 64 * h:C + 64 * (h + 1)] for h in hs])])
        v_rows = np.concatenate([W_attn[2 * C + 64 * h:2 * C + 64 * (h + 1)] for h in hs], 0)
        WvT = v_rows.T                                        # [1024, 256]
        bv = np.concatenate([b_attn[2 * C + 64 * h:2 * C + 64 * (h + 1)] for h in hs])
        WpT = np.concatenate([W_proj[:, 64 * h:64 * (h + 1)] for h in hs], 1).T  # [256,1024]
        in_maps.append({
            "xT": np.ascontiguousarray(
                xT.reshape(NCT, 128, TL).transpose(1, 0, 2)),
            "wqkT": np.ascontiguousarray(
                WqkT.astype(bf).reshape(NCT, 128, 512).transpose(1, 0, 2)),
            "wvT": np.ascontiguousarray(
                WvT.astype(bf).reshape(NCT, 128, 256).transpose(1, 0, 2)),
            "bqk": np.ascontiguousarray(bqk.reshape(4, 128).T.astype(np.float32)),
            "bv": np.ascontiguousarray(
                np.broadcast_to(bv[None, :].astype(np.float32), (128, 256))),
            "rope": ropeT,
            "masks": masks,
            "wpT": np.ascontiguousarray(
                WpT.astype(bf).reshape(2, 128, 1024).transpose(1, 0, 2)),
        })
    return in_maps


def kernel(x, W_attn, b_attn, W_proj, b_proj):
    if "nc" not in _CACHE:
        _CACHE["nc"] = _build_nc()
    nc = _CACHE["nc"]
    in_maps = _host_inputs(x, W_attn, b_attn, W_proj, b_proj)
    res = run_bass_kernel_spmd(nc, in_maps, list(range(8)), trace=TRACE)
    _CACHE["last"] = res
    y = np.zeros((B, T, C), np.float32)
    for c in range(8):
        y[c // 4] += res.results[c]["yT"].T
    y += b_proj.astype(np.float32)
    return y


# revision 6
# speedup vs baseline: 1.0501x; 1.0501x over previous
"""Causal self-attention (B=2, T=2048, C=1024, NH=16, HD=64) on 8 TRN2 cores.

Sharding: core c -> batch b = c//4, head group j = c%4 (4 heads: 4j..4j+3).
Each core computes its batch's QKV projection for its 4 heads, rope, causal
flash-style attention in S^T layout (k on partitions, q on free dim), and a
partial output projection y_part^T = Wp_slice^T.T @ attT. Host sums the 4
per-batch partials and adds b_proj.

Device layouts (per core, t = 2048 tokens of its batch):
  xT   [128, 8, 2048]  bf16   x[b].T tiled over 8 c-tiles
  q/k  [128, 2, 2048]  bf16   head-pair dims on partitions, rope applied
  v    [128, 64, 65]   bf16   [tok-part, u=(tt,hp,h), 64 dims + ones col]
  S^T  psum [128, 2048] f32   4 tiles: [kt0h0|kt0h1|kt1h0|kt1h1]
  P^T  [128, 2048] bf16       exp(S^T/8), causal-masked (mask mul on GpSimd)
  PV   psum [65, 512]         rows 0-63 att^T, row 64 rowsum (ones col)
  attT [128, 2, 2048] f32 -> normalized bf16 -> y^T = WpT.T @ attbf
"""
import numpy as np
import ml_dtypes
from contextlib import ExitStack

import concourse.bass as bass
import concourse.mybir as mybir
import concourse.tile as tile
from concourse import bacc
from concourse.bass_utils import run_bass_kernel_spmd

F32 = mybir.dt.float32
BF16 = mybir.dt.bfloat16
AF = mybir.ActivationFunctionType
ALU = mybir.AluOpType

B, T, C = 2, 2048, 1024
NH, HD = 16, 64
TL = 2048          # per-core token count (one batch)
NCT = C // 128     # 8 contraction tiles
NTC = TL // 512    # 4 t-chunks of 512
NTT = TL // 128    # 16 token tiles of 128

TRACE = False      # set by test harness for profiling runs
_CACHE = {}


def _build_nc():
    nc = bacc.Bacc("TRN2", target_bir_lowering=False, debug=False)
    xT_d = nc.dram_tensor("xT", [128, NCT, TL], BF16, kind="ExternalInput").ap()
    wqk_d = nc.dram_tensor("wqkT", [128, NCT, 512], BF16, kind="ExternalInput").ap()
    wv_d = nc.dram_tensor("wvT", [128, NCT, 256], BF16, kind="ExternalInput").ap()
    bqk_d = nc.dram_tensor("bqk", [128, 4], F32, kind="ExternalInput").ap()
    bv_d = nc.dram_tensor("bv", [128, 256], F32, kind="ExternalInput").ap()
    rope_d = nc.dram_tensor("rope", [128, TL], F32, kind="ExternalInput").ap()
    masks_d = nc.dram_tensor("masks", [128, 4, 1024], BF16, kind="ExternalInput").ap()
    wp_d = nc.dram_tensor("wpT", [128, 2, 1024], BF16, kind="ExternalInput").ap()
    yT_d = nc.dram_tensor("yT", [1024, TL], F32, kind="ExternalOutput").ap()
    rs_dram = nc.dram_tensor("rs_scratch", [4, TL], F32)

    with tile.TileContext(nc) as tc, ExitStack() as ctx:
        sb = ctx.enter_context(tc.tile_pool(name="sb", bufs=1))
        ptp = ctx.enter_context(tc.tile_pool(name="ptp", bufs=4))
        ytp = ctx.enter_context(tc.tile_pool(name="ytp", bufs=8))

        xT = sb.tile([128, NCT, TL], BF16)
        wqk = sb.tile([128, NCT, 512], BF16)
        wv = sb.tile([128, NCT, 256], BF16)
        bqk = sb.tile([128, 4], F32)
        bv = sb.tile([128, 256], F32)
        rope = sb.tile([128, TL], F32)
        masks = sb.tile([128, 4, 1024], BF16)
        wp = sb.tile([128, 2, 1024], BF16)
        q_sb = sb.tile([128, 2, TL], BF16)
        k_sb = sb.tile([128, 2, TL], BF16)
        v_sb = sb.tile([128, 4 * NTT, 65], BF16)
        attT = sb.tile([128, 2, TL], F32)
        attbf = sb.tile([128, 2, TL], BF16)
        rs_sb = sb.tile([128, TL], F32)   # rowsum for unit u at partition 32*u
        rsr_sb = sb.tile([128, TL], F32)
        rsb_sb = sb.tile([128, 2, TL], F32)

        # small/weight DMAs first so phase B isn't gated on the x stream
        nc.sync.dma_start(out=wqk, in_=wqk_d)
        nc.sync.dma_start(out=bqk, in_=bqk_d)
        nc.sync.dma_start(out=rope, in_=rope_d)
        nc.sync.dma_start(out=wv, in_=wv_d)
        nc.sync.dma_start(out=bv, in_=bv_d)
        nc.sync.dma_start(out=masks, in_=masks_d)
        nc.sync.dma_start(out=wp, in_=wp_d)
        for ct in range(NCT):
            nc.sync.dma_start(out=xT[:, ct, :], in_=xT_d[:, ct, :])
        nc.vector.memset(v_sb[:, :, 64:65], 1.0)
        nc.gpsimd.memset(rs_sb, 1.0)  # keep unused partitions finite for recip

        def phase_b(ms, pb):
            """QK projection for m-tiles in ms: psum -> (+bias)*rope -> q/k."""
            for m in ms:
                pss = [pb.tile([128, 512], F32, tag=f"pb{i}", name=f"pb_{m}_{i}")
                       for i in range(NTC)]
                for ct in range(NCT):
                    lhsT = wqk[:, ct, m * 128:(m + 1) * 128]
                    for tc4 in range(NTC):
                        nc.tensor.matmul(
                            pss[tc4], lhsT, xT[:, ct, tc4 * 512:(tc4 + 1) * 512],
                            start=(ct == 0), stop=(ct == NCT - 1),
                            skip_group_check=True)
                dest = q_sb if m < 2 else k_sb
                hp = m % 2
                for tc4 in range(NTC):
                    sl = slice(tc4 * 512, (tc4 + 1) * 512)
                    nc.vector.scalar_tensor_tensor(
                        out=dest[:, hp, sl], in0=pss[tc4], scalar=bqk[:, m:m + 1],
                        in1=rope[:, sl], op0=ALU.add, op1=ALU.mult)

        def phase_c(pc):
            """V projection, natural layout, bias added, ones col preset."""
            for tt in range(NTT):
                ps = pc.tile([128, 256], F32, tag="pc", name=f"pc_{tt}")
                for ct in range(NCT):
                    nc.tensor.matmul(
                        ps, xT[:, ct, tt * 128:(tt + 1) * 128], wv[:, ct, :],
                        start=(ct == 0), stop=(ct == NCT - 1),
                        skip_group_check=True)
                # one fused add for all 4 units of this token tile:
                # psum cols (hp*128+h*64) map to v_sb units u=4tt+2hp+h in order
                nc.vector.tensor_add(
                    v_sb[:, 4 * tt:4 * tt + 4, 0:64],
                    ps.rearrange("p (a b) -> p a b", a=4),
                    bv.rearrange("p (a b) -> p a b", a=4))

        def phase_d(hp, pds, pdv):
            """Attention for head pair hp."""
            for qc in range(4):
                qsl = slice(qc * 512, (qc + 1) * 512)
                pv = [pdv.tile([65, 512], F32, tag=f"pv{h}", name=f"pv_{qc}_{hp}_{h}")
                      for h in range(2)]
                n_kt = 4 * (qc + 1)
                for g in range(n_kt // 2):
                    sps = pds.tile([128, 2048], F32, tag="sps", name=f"sps_{qc}_{hp}_{g}")
                    for i in range(2):
                        kt = 2 * g + i
                        ksl = slice(kt * 128, (kt + 1) * 128)
                        for h in range(2):
                            hsl = slice(h * 64, (h + 1) * 64)
                            nc.tensor.matmul(
                                sps[:, (i * 2 + h) * 512:(i * 2 + h + 1) * 512],
                                k_sb[hsl, hp, ksl], q_sb[hsl, hp, qsl])
                    pt = ptp.tile([128, 2048], BF16, tag="pt", name=f"pt_{qc}_{hp}_{g}")
                    nc.scalar.activation(pt, sps, AF.Exp, bias=0.0, scale=0.125)
                    for i in range(2):
                        kt = 2 * g + i
                        if kt >= 4 * qc:  # partial (diagonal) tile: mask on POOL
                            d = kt - 4 * qc
                            nc.gpsimd.tensor_mul(
                                pt[:, i * 1024:(i + 1) * 1024],
                                pt[:, i * 1024:(i + 1) * 1024],
                                masks[:, d, :])
                    for i in range(2):
                        kt = 2 * g + i
                        for h in range(2):
                            u = (kt * 2 + hp) * 2 + h
                            nc.tensor.matmul(
                                pv[h], v_sb[:, u, :],
                                pt[:, (i * 2 + h) * 512:(i * 2 + h + 1) * 512],
                                start=(kt == 0), stop=(kt == n_kt - 1),
                                skip_group_check=True)
                for h in range(2):
                    u4 = 2 * hp + h
                    nc.vector.tensor_copy(
                        attT[h * 64:(h + 1) * 64, hp, qsl], pv[h][0:64, :])
                    nc.vector.tensor_copy(
                        rs_sb[32 * u4:32 * u4 + 1, qsl], pv[h][64:65, :])

        with tc.tile_pool(name="pb", bufs=1, space="PSUM") as pb, \
             tc.tile_pool(name="pc", bufs=2, space="PSUM") as pc:
            phase_b((2, 0), pb)   # k then q for head pair 0
            phase_c(pc)
            phase_b((3, 1), pb)   # head pair 1

        with tc.tile_pool(name="pds", bufs=1, space="PSUM") as pds, \
             tc.tile_pool(name="pdv", bufs=2, space="PSUM") as pdv:
            phase_d(0, pds, pdv)
            phase_d(1, pds, pdv)

        # ---- rowsum normalize: fast reciprocal, DRAM-bounce broadcast ----
        nc.vector.reciprocal_approx_fast(rsr_sb, rs_sb)
        for u4 in range(4):
            nc.sync.dma_start(
                out=rs_dram[u4:u4 + 1, :], in_=rsr_sb[32 * u4:32 * u4 + 1, :])
        for hp in range(2):
            for h in range(2):
                u4 = 2 * hp + h
                bc_ap = bass.AP(tensor=rs_dram, offset=u4 * TL, ap=[[0, 64], [1, TL]])
                nc.sync.dma_start(out=rsb_sb[h * 64:(h + 1) * 64, hp, :], in_=bc_ap)
        for hp in range(2):
            nc.vector.tensor_mul(attbf[:, hp, :], attT[:, hp, :], rsb_sb[:, hp, :])

        # ---- Phase E: yT = WpT.T @ attbf ----
        with tc.tile_pool(name="pe", bufs=4, space="PSUM") as pe:
            for mt in range(8):
                pss = [pe.tile([128, 512], F32, tag=f"pe{i % 2}", name=f"pe_{mt}_{i}")
                       for i in range(NTC)]
                for hp in range(2):
                    lhsT = wp[:, hp, mt * 128:(mt + 1) * 128]
                    for tc4 in range(NTC):
                        nc.tensor.matmul(
                            pss[tc4], lhsT, attbf[:, hp, tc4 * 512:(tc4 + 1) * 512],
                            start=(hp == 0), stop=(hp == 1),
                            skip_group_check=True)
                for tc4 in range(NTC):
                    yt = ytp.tile([128, 512], F32, tag="yt", name=f"yt_{mt}_{tc4}")
                    if tc4 % 2 == 0:
                        nc.vector.tensor_copy(yt, pss[tc4])
                    else:
                        nc.scalar.copy(yt, pss[tc4])
                    nc.sync.dma_start(
                        out=yT_d[mt * 128:(mt + 1) * 128, tc4 * 512:(tc4 + 1) * 512],
                        in_=yt)
    nc.compile()
    return nc


def _rope_T():
    theta = 1.0 / (10000.0 ** (2.0 * np.arange(0, HD // 2, dtype=np.float32) / HD))
    seq = np.arange(1, T + 1, dtype=np.float32)
    ang = np.einsum('n,d->nd', seq, theta)
    ang = np.concatenate([ang, ang], axis=-1)
    f = (np.cos(ang) + np.sin(ang)).astype(np.float32)  # [T, 64]
    return np.concatenate([f.T, f.T], axis=0)           # [128, T]


def _host_inputs(x, W_attn, b_attn, W_proj, b_proj):
    bf = ml_dtypes.bfloat16
    ropeT = _rope_T()
    masks = np.empty((128, 4, 1024), dtype=bf)
    kp = np.arange(128)[:, None]
    qf = np.arange(512)[None, :]
    for d in range(4):
        m = ((kp + 128 * d) <= qf).astype(np.float32)
        masks[:, d, :] = np.concatenate([m, m], axis=1).astype(bf)

    in_maps = []
    for c in range(8):
        b, j = divmod(c, 4)
        hs = [4 * j + i for i in range(4)]
        xT = np.ascontiguousarray(x[b].T).astype(bf)          # [1024, TL]
        q_rows = np.concatenate([W_attn[64 * h:64 * (h + 1)] for h in hs], 0)
        k_rows = np.concatenate([W_attn[C + 64 * h:C + 64 * (h + 1)] for h in hs], 0)
        WqkT = np.concatenate([q_rows, k_rows], 0).T          # [1024, 512]
        bqk = np.concatenate(
            [np.concatenate([b_attn[64 * h:64 * (h + 1)] for h in hs]),
             np.concatenate([b_attn[C + 64 * h:C + 64 * (h + 1)] for h in hs])])
        v_rows = np.concatenate([W_attn[2 * C + 64 * h:2 * C + 64 * (h + 1)] for h in hs], 0)
        WvT = v_rows.T                                        # [1024, 256]
        bv = np.concatenate([b_attn[2 * C + 64 * h:2 * C + 64 * (h + 1)] for h in hs])
        WpT = np.concatenate([W_proj[:, 64 * h:64 * (h + 1)] for h in hs], 1).T  # [256,1024]
        in_maps.append({
            "xT": np.ascontiguousarray(
                xT.reshape(NCT, 128, TL).transpose(1, 0, 2)),
            "wqkT": np.ascontiguousarray(
                WqkT.astype(bf).reshape(NCT, 128, 512).transpose(1, 0, 2)),
            "wvT": np.ascontiguousarray(
                WvT.astype(bf).reshape(NCT, 128, 256).transpose(1, 0, 2)),
            "bqk": np.ascontiguousarray(bqk.reshape(4, 128).T.astype(np.float32)),
            "bv": np.ascontiguousarray(
                np.broadcast_to(bv[None, :].astype(np.float32), (128, 256))),
            "rope": ropeT,
            "masks": masks,
            "wpT": np.ascontiguousarray(
                WpT.astype(bf).reshape(2, 128, 1024).transpose(1, 0, 2)),
        })
    return in_maps


def kernel(x, W_attn, b_attn, W_proj, b_proj):
    if "nc" not in _CACHE:
        _CACHE["nc"] = _build_nc()
    nc = _CACHE["nc"]
    in_maps = _host_inputs(x, W_attn, b_attn, W_proj, b_proj)
    res = run_bass_kernel_spmd(nc, in_maps, list(range(8)), trace=TRACE)
    _CACHE["last"] = res
    y = np.zeros((B, T, C), np.float32)
    for c in range(8):
        y[c // 4] += res.results[c]["yT"].T
    y += b_proj.astype(np.float32)
    return y


# revision 7
# speedup vs baseline: 1.4558x; 1.3863x over previous
"""Causal self-attention (B=2, T=2048, C=1024, NH=16, HD=64) on 8 TRN2 cores.

Sharding: core c -> batch b = c//4, head group j = c%4 (4 heads: 4j..4j+3).
Each core computes its batch's QKV projection for its 4 heads, rope, causal
flash-style attention in S^T layout (k on partitions, q on free dim), and a
partial output projection y_part^T = Wp_slice^T.T @ attT. Host sums the 4
per-batch partials and adds b_proj.

Device layouts (per core, t = 2048 tokens of its batch):
  xT   [128, 8, 2048]  bf16   x[b].T tiled over 8 c-tiles
  q/k  [128, 2, 2048]  bf16   head-pair dims on partitions, rope applied
  v    [128, 64, 65]   bf16   [tok-part, u=(tt,hp,h), 64 dims + ones col]
  S^T  psum [128, 2048] f32   4 tiles: [kt0h0|kt0h1|kt1h0|kt1h1]
  P^T  [128, 2048] bf16       exp(S^T/8), causal-masked (mask mul on GpSimd)
  PV   psum [65, 512]         rows 0-63 att^T, row 64 rowsum (ones col)
  attT [128, 2, 2048] f32 -> normalized bf16 -> y^T = WpT.T @ attbf
"""
import numpy as np
import ml_dtypes
from contextlib import ExitStack

import concourse.bass as bass
import concourse.mybir as mybir
import concourse.tile as tile
from concourse import bacc
from concourse.bass_utils import run_bass_kernel_spmd

F32 = mybir.dt.float32
BF16 = mybir.dt.bfloat16
AF = mybir.ActivationFunctionType
ALU = mybir.AluOpType

B, T, C = 2, 2048, 1024
NH, HD = 16, 64
TL = 2048          # per-core token count (one batch)
NCT = C // 128     # 8 contraction tiles
NTC = TL // 512    # 4 t-chunks of 512
NTT = TL // 128    # 16 token tiles of 128

TRACE = False      # set by test harness for profiling runs
_CACHE = {}


def _build_nc():
    nc = bacc.Bacc("TRN2", target_bir_lowering=False, debug=False)
    xT_d = nc.dram_tensor("xT", [128, NCT, TL], BF16, kind="ExternalInput").ap()
    wqk_d = nc.dram_tensor("wqkT", [128, NCT, 512], BF16, kind="ExternalInput").ap()
    wv_d = nc.dram_tensor("wvT", [128, NCT, 256], BF16, kind="ExternalInput").ap()
    bqk_d = nc.dram_tensor("bqk", [128, 4], F32, kind="ExternalInput").ap()
    bv_d = nc.dram_tensor("bv", [128, 256], F32, kind="ExternalInput").ap()
    rope_d = nc.dram_tensor("rope", [128, TL], F32, kind="ExternalInput").ap()
    masks_d = nc.dram_tensor("masks", [128, 4, 1024], BF16, kind="ExternalInput").ap()
    wp_d = nc.dram_tensor("wpT", [128, 2, 1024], BF16, kind="ExternalInput").ap()
    yT_d = nc.dram_tensor("yT", [1024, TL], F32, kind="ExternalOutput").ap()
    rs_dram = nc.dram_tensor("rs_scratch", [4, TL], F32)

    with tile.TileContext(nc) as tc, ExitStack() as ctx:
        sb = ctx.enter_context(tc.tile_pool(name="sb", bufs=1))
        ptp = ctx.enter_context(tc.tile_pool(name="ptp", bufs=6))
        ytp = ctx.enter_context(tc.tile_pool(name="ytp", bufs=8))

        xT = sb.tile([128, NCT, TL], BF16)
        wqk = sb.tile([128, NCT, 512], BF16)
        wv = sb.tile([128, NCT, 256], BF16)
        bqk = sb.tile([128, 4], F32)
        bv = sb.tile([128, 256], F32)
        rope = sb.tile([128, TL], F32)
        masks = sb.tile([128, 4, 1024], BF16)
        wp = sb.tile([128, 2, 1024], BF16)
        q_sb = sb.tile([128, 2, TL], BF16)
        k_sb = sb.tile([128, 2, TL], BF16)
        v_sb = sb.tile([128, 4 * NTT, 65], BF16)
        attT = sb.tile([128, 2, TL], F32)
        attbf = sb.tile([128, 2, TL], BF16)
        rs_sb = sb.tile([128, TL], F32)   # rowsum for unit u at partition 32*u
        rsr_sb = sb.tile([128, TL], F32)
        rsb_sb = sb.tile([128, 2, TL], F32)

        # phase-B-critical DMAs first; spread across DGE rings (sync/gpsimd/
        # scalar issue queues) so the x stream and small tensors run parallel
        nc.gpsimd.dma_start(out=wqk, in_=wqk_d)
        nc.gpsimd.dma_start(out=bqk, in_=bqk_d)
        for ct in range(NCT):
            nc.sync.dma_start(out=xT[:, ct, :], in_=xT_d[:, ct, :])
        nc.gpsimd.dma_start(out=rope, in_=rope_d)
        nc.scalar.dma_start(out=wv, in_=wv_d)
        nc.scalar.dma_start(out=bv, in_=bv_d)
        nc.gpsimd.dma_start(out=masks, in_=masks_d)
        nc.scalar.dma_start(out=wp, in_=wp_d)
        nc.vector.memset(v_sb[:, :, 64:65], 1.0)
        nc.gpsimd.memset(rs_sb, 1.0)  # keep unused partitions finite for recip

        def phase_b(ms, pb):
            """QK projection for m-tiles in ms: psum -> (+bias)*rope -> q/k."""
            for m in ms:
                pss = [pb.tile([128, 512], F32, tag=f"pb{i}", name=f"pb_{m}_{i}")
                       for i in range(NTC)]
                for ct in range(NCT):
                    lhsT = wqk[:, ct, m * 128:(m + 1) * 128]
                    for tc4 in range(NTC):
                        nc.tensor.matmul(
                            pss[tc4], lhsT, xT[:, ct, tc4 * 512:(tc4 + 1) * 512],
                            start=(ct == 0), stop=(ct == NCT - 1),
                            skip_group_check=True)
                dest = q_sb if m < 2 else k_sb
                hp = m % 2
                for tc4 in range(NTC):
                    sl = slice(tc4 * 512, (tc4 + 1) * 512)
                    nc.vector.scalar_tensor_tensor(
                        out=dest[:, hp, sl], in0=pss[tc4], scalar=bqk[:, m:m + 1],
                        in1=rope[:, sl], op0=ALU.add, op1=ALU.mult)

        def phase_c(pc):
            """V projection, natural layout, bias added, ones col preset."""
            for tt in range(NTT):
                ps = pc.tile([128, 256], F32, tag="pc", name=f"pc_{tt}")
                for ct in range(NCT):
                    nc.tensor.matmul(
                        ps, xT[:, ct, tt * 128:(tt + 1) * 128], wv[:, ct, :],
                        start=(ct == 0), stop=(ct == NCT - 1),
                        skip_group_check=True)
                # one fused add for all 4 units of this token tile:
                # psum cols (hp*128+h*64) map to v_sb units u=4tt+2hp+h in order
                nc.vector.tensor_add(
                    v_sb[:, 4 * tt:4 * tt + 4, 0:64],
                    ps.rearrange("p (a b) -> p a b", a=4),
                    bv.rearrange("p (a b) -> p a b", a=4))

        def phase_d(hp, pds, pdv):
            """Attention for head pair hp. One kt per S^T/exp group so the
            3-buffered psum keeps PE(S) / ACT(exp) / PE(PV) overlapped."""
            for qc in range(4):
                qsl = slice(qc * 512, (qc + 1) * 512)
                pv = [pdv.tile([65, 512], F32, tag=f"pv{h}", name=f"pv_{qc}_{hp}_{h}")
                      for h in range(2)]
                n_kt = 4 * (qc + 1)
                for kt in range(n_kt):
                    ksl = slice(kt * 128, (kt + 1) * 128)
                    sps = pds.tile([128, 1024], F32, tag="sps", name=f"sps_{qc}_{hp}_{kt}")
                    for h in range(2):
                        hsl = slice(h * 64, (h + 1) * 64)
                        nc.tensor.matmul(
                            sps[:, h * 512:(h + 1) * 512],
                            k_sb[hsl, hp, ksl], q_sb[hsl, hp, qsl])
                    pt = ptp.tile([128, 1024], BF16, tag="pt", name=f"pt_{qc}_{hp}_{kt}")
                    nc.scalar.activation(pt, sps, AF.Exp, bias=0.0, scale=0.125)
                    if kt >= 4 * qc:  # partial (diagonal) tile: mask both heads
                        d = kt - 4 * qc
                        nc.vector.tensor_mul(pt, pt, masks[:, d, :])
                    for h in range(2):
                        u = (kt * 2 + hp) * 2 + h
                        nc.tensor.matmul(
                            pv[h], v_sb[:, u, :], pt[:, h * 512:(h + 1) * 512],
                            start=(kt == 0), stop=(kt == n_kt - 1),
                            skip_group_check=True)
                for h in range(2):
                    u4 = 2 * hp + h
                    nc.vector.tensor_copy(
                        attT[h * 64:(h + 1) * 64, hp, qsl], pv[h][0:64, :])
                    nc.vector.tensor_copy(
                        rs_sb[32 * u4:32 * u4 + 1, qsl], pv[h][64:65, :])

        with tc.tile_pool(name="pb", bufs=1, space="PSUM") as pb, \
             tc.tile_pool(name="pc", bufs=2, space="PSUM") as pc:
            phase_b((2, 0), pb)   # k then q for head pair 0
            phase_c(pc)
            phase_b((3, 1), pb)   # head pair 1

        with tc.tile_pool(name="pds", bufs=3, space="PSUM") as pds, \
             tc.tile_pool(name="pdv", bufs=1, space="PSUM") as pdv:
            phase_d(0, pds, pdv)
            phase_d(1, pds, pdv)

        # ---- rowsum normalize: fast reciprocal, DRAM-bounce broadcast ----
        nc.vector.reciprocal_approx_fast(rsr_sb, rs_sb)
        for u4 in range(4):
            nc.sync.dma_start(
                out=rs_dram[u4:u4 + 1, :], in_=rsr_sb[32 * u4:32 * u4 + 1, :])
        for hp in range(2):
            for h in range(2):
                u4 = 2 * hp + h
                bc_ap = bass.AP(tensor=rs_dram, offset=u4 * TL, ap=[[0, 64], [1, TL]])
                nc.sync.dma_start(out=rsb_sb[h * 64:(h + 1) * 64, hp, :], in_=bc_ap)
        for hp in range(2):
            nc.vector.tensor_mul(attbf[:, hp, :], attT[:, hp, :], rsb_sb[:, hp, :])

        # ---- Phase E: yT = WpT.T @ attbf ----
        with tc.tile_pool(name="pe", bufs=4, space="PSUM") as pe:
            for mt in range(8):
                pss = [pe.tile([128, 512], F32, tag=f"pe{i % 2}", name=f"pe_{mt}_{i}")
                       for i in range(NTC)]
                for hp in range(2):
                    lhsT = wp[:, hp, mt * 128:(mt + 1) * 128]
                    for tc4 in range(NTC):
                        nc.tensor.matmul(
                            pss[tc4], lhsT, attbf[:, hp, tc4 * 512:(tc4 + 1) * 512],
                            start=(hp == 0), stop=(hp == 1),
                            skip_group_check=True)
                for tc4 in range(NTC):
                    yt = ytp.tile([128, 512], F32, tag="yt", name=f"yt_{mt}_{tc4}")
                    if tc4 % 2 == 0:
                        nc.vector.tensor_copy(yt, pss[tc4])
                    else:
                        nc.scalar.copy(yt, pss[tc4])
                    nc.sync.dma_start(
                        out=yT_d[mt * 128:(mt + 1) * 128, tc4 * 512:(tc4 + 1) * 512],
                        in_=yt)
    nc.compile()
    return nc


def _rope_T():
    theta = 1.0 / (10000.0 ** (2.0 * np.arange(0, HD // 2, dtype=np.float32) / HD))
    seq = np.arange(1, T + 1, dtype=np.float32)
    ang = np.einsum('n,d->nd', seq, theta)
    ang = np.concatenate([ang, ang], axis=-1)
    f = (np.cos(ang) + np.sin(ang)).astype(np.float32)  # [T, 64]
    return np.concatenate([f.T, f.T], axis=0)           # [128, T]


def _host_inputs(x, W_attn, b_attn, W_proj, b_proj):
    bf = ml_dtypes.bfloat16
    ropeT = _rope_T()
    masks = np.empty((128, 4, 1024), dtype=bf)
    kp = np.arange(128)[:, None]
    qf = np.arange(512)[None, :]
    for d in range(4):
        m = ((kp + 128 * d) <= qf).astype(np.float32)
        masks[:, d, :] = np.concatenate([m, m], axis=1).astype(bf)

    in_maps = []
    for c in range(8):
        b, j = divmod(c, 4)
        hs = [4 * j + i for i in range(4)]
        xT = np.ascontiguousarray(x[b].T).astype(bf)          # [1024, TL]
        q_rows = np.concatenate([W_attn[64 * h:64 * (h + 1)] for h in hs], 0)
        k_rows = np.concatenate([W_attn[C + 64 * h:C + 64 * (h + 1)] for h in hs], 0)
        WqkT = np.concatenate([q_rows, k_rows], 0).T          # [1024, 512]
        bqk = np.concatenate(
            [np.concatenate([b_attn[64 * h:64 * (h + 1)] for h in hs]),
             np.concatenate([b_attn[C + 64 * h:C + 64 * (h + 1)] for h in hs])])
        v_rows = np.concatenate([W_attn[2 * C + 64 * h:2 * C + 64 * (h + 1)] for h in hs], 0)
        WvT = v_rows.T                                        # [1024, 256]
        bv = np.concatenate([b_attn[2 * C + 64 * h:2 * C + 64 * (h + 1)] for h in hs])
        WpT = np.concatenate([W_proj[:, 64 * h:64 * (h + 1)] for h in hs], 1).T  # [256,1024]
        in_maps.append({
            "xT": np.ascontiguousarray(
                xT.reshape(NCT, 128, TL).transpose(1, 0, 2)),
            "wqkT": np.ascontiguousarray(
                WqkT.astype(bf).reshape(NCT, 128, 512).transpose(1, 0, 2)),
            "wvT": np.ascontiguousarray(
                WvT.astype(bf).reshape(NCT, 128, 256).transpose(1, 0, 2)),
            "bqk": np.ascontiguousarray(bqk.reshape(4, 128).T.astype(np.float32)),
            "bv": np.ascontiguousarray(
                np.broadcast_to(bv[None, :].astype(np.float32), (128, 256))),
            "rope": ropeT,
            "masks": masks,
            "wpT": np.ascontiguousarray(
                WpT.astype(bf).reshape(2, 128, 1024).transpose(1, 0, 2)),
        })
    return in_maps


def kernel(x, W_attn, b_attn, W_proj, b_proj):
    if "nc" not in _CACHE:
        _CACHE["nc"] = _build_nc()
    nc = _CACHE["nc"]
    in_maps = _host_inputs(x, W_attn, b_attn, W_proj, b_proj)
    res = run_bass_kernel_spmd(nc, in_maps, list(range(8)), trace=TRACE)
    _CACHE["last"] = res
    y = np.zeros((B, T, C), np.float32)
    for c in range(8):
        y[c // 4] += res.results[c]["yT"].T
    y += b_proj.astype(np.float32)
    return y
